# revision 1
# baseline (speedup 1.0000x reference)
"""CapsuleLayer dynamic-routing kernel for Trainium2 (8 NeuronCores).

Problem: B=256, I=2048, D=8 input capsules -> J=10, E=16 output capsules,
3 routing iterations.  Output = concat([v2, c2], axis=-1) -> [B, J, E+I].

Sharding: pure data parallelism over batch (32 batches/core), W replicated.

Per-core design (u_hat is NEVER materialized):
  s-steps:  s[b,j,e] = sum_{i,d} X[b,j,i,d] * W[j,i,e,d],  X = c (.) inputs
            PE matmuls, K=i (128-chunks), PSUM-accumulated over (chunk, d).
            Stationary wf[(i),(j,e)] slices, moving X[(i),(j,b)] -> diagonal
            j==j' entries of out[(j,e),(j',b)] are the result.
  t-steps:  t[b,j,i] = sum_e v[b,j,e] u_hat[b,j,i,e] computed as
            M1:   Y[(i,d), b]   = sum_e wm1[e,(i,d)]^T v[e,b]   (per j, PE)
            evac: Z[(i,d),(j,b)] = Y (.) inputs                  (DVE, PSUM->SBUF)
            M2:   blog[i, (j,b)] += ones_blkdiag^T Z             (PE, sums d)
  softmax over j without max-subtraction (logits are O(few)), ACT exp + DVE.

Layouts (i = ch*128 + p for the i-partition tensors; k = i*8 + d for (i,d)):
  wf    [128, 16, 8, 160]   wf[p,ch,d,j*16+e] = W[j, ch*128+p, e, d]
  wm1   [16, 10, 16384]     wm1[e,j,i*8+d]    = W[j, i, e, d]
  inp_i [128, 16, 32, 8]    inp_i[p,ch,b,d]   = inputs[b0+b, ch*128+p, d]
  inp_id[128, 128, 32]      inp_id[q,g,b]     = inputs[b0+b, g*16+q//8, q%8]
  ones  [128, 8, 128]       ones[q,gq,m]      = (m == 16*gq + q//8)
"""

import numpy as np

B, I, D, J, E = 256, 2048, 8, 10, 16
NCORES = 8
BL = B // NCORES          # 32 batches per core
NCH = I // 128            # 16 i-chunks of 128
NG = (I * D) // 128       # 128 (i,d)-groups of 128
JE = J * E                # 160
JB = J * BL               # 320  (j,b) column count
OUTW = E + I              # 2064
EPS = 1e-7

_PROGRAM = None


def _host_prep(inputs, W):
    """Build all DRAM-side arrays. Returns (shared dict, per-core list)."""
    W = np.ascontiguousarray(W, dtype=np.float32)
    inputs = np.ascontiguousarray(inputs, dtype=np.float32)

    # wf[p, ch, d, j*16+e] = W[j, ch*128+p, e, d]
    wf = W.transpose(1, 3, 0, 2).reshape(NCH, 128, D, JE)  # [ch,p,d,(j,e)] wait
    # W.transpose(1,3,0,2): [I, D, J, E] -> index [i, d, j, e]
    # reshape(NCH,128,D,J*E) splits i -> (ch, p): [ch, p, d, (j,e)]
    wf = np.ascontiguousarray(wf.transpose(1, 0, 2, 3))   # [p, ch, d, (j,e)]

    # wm1[e, j, i*8+d] = W[j, i, e, d]
    wm1 = np.ascontiguousarray(W.transpose(2, 0, 1, 3).reshape(E, J, I * D))

    # ones[q, gq, m] = 1 iff m == 16*gq + q//8
    ones = np.zeros((128, 8, 128), dtype=np.float32)
    q = np.arange(128)
    for gq in range(8):
        ones[q, gq, 16 * gq + q // 8] = 1.0

    shared = {"wf": wf, "wm1": wm1, "ones": ones}

    per_core = []
    for m in range(NCORES):
        sl = inputs[m * BL:(m + 1) * BL]                  # [32, 2048, 8]
        # inp_i[p, ch, b, d] = sl[b, ch*128+p, d]
        inp_i = np.ascontiguousarray(
            sl.reshape(BL, NCH, 128, D).transpose(2, 1, 0, 3))
        # inp_id[q, g, b] = sl[b, g*16 + q//8, q%8]
        inp_id = np.ascontiguousarray(
            sl.reshape(BL, NG, 16, 8).transpose(2, 3, 1, 0).reshape(128, NG, BL))
        per_core.append({"inp_i": inp_i, "inp_id": inp_id})
    return shared, per_core


def _build_program(debug=False):
    from contextlib import ExitStack
    import concourse.mybir as mybir
    from concourse import bacc
    from concourse.tile import TileContext

    f32 = mybir.dt.float32
    nc = bacc.Bacc()

    wf_d = nc.dram_tensor("wf", [128, NCH, D, JE], f32, kind="ExternalInput")
    wm1_d = nc.dram_tensor("wm1", [E, J, I * D], f32, kind="ExternalInput")
    ones_d = nc.dram_tensor("ones", [128, 8, 128], f32, kind="ExternalInput")
    inpi_d = nc.dram_tensor("inp_i", [128, NCH, BL, D], f32, kind="ExternalInput")
    inpid_d = nc.dram_tensor("inp_id", [128, NG, BL], f32, kind="ExternalInput")
    out_d = nc.dram_tensor("out", [BL, J, OUTW], f32, kind="ExternalOutput")

    _kernel_body.debug_tensors = {}
    if debug:
        _kernel_body.debug_tensors = {
            "s0": nc.dram_tensor("dbg_s0", [BL, J, 32], f32, kind="ExternalOutput"),
            "v0": nc.dram_tensor("dbg_v0", [E, J, BL], f32, kind="ExternalOutput"),
            "blog0": nc.dram_tensor("dbg_blog0", [128, NCH, J, BL], f32,
                                    kind="ExternalOutput"),
            "c1": nc.dram_tensor("dbg_c1", [128, NCH, J, BL], f32,
                                 kind="ExternalOutput"),
            "x0": nc.dram_tensor("dbg_x0", [128, D, J, BL], f32,
                                 kind="ExternalOutput"),
            "psa0": nc.dram_tensor("dbg_psa0", [128, 8, BL], f32,
                                   kind="ExternalOutput"),
            "psb0": nc.dram_tensor("dbg_psb0", [32, 2, BL], f32,
                                   kind="ExternalOutput"),
        }

    with ExitStack() as ctx:
        tc = ctx.enter_context(TileContext(nc))
        _kernel_body(ctx, tc, wf_d, wm1_d, ones_d, inpi_d, inpid_d, out_d)
    nc.compile()
    return nc


def _kernel_body(ctx, tc, wf_d, wm1_d, ones_d, inpi_d, inpid_d, out_d):
    import concourse.bass as bass
    import concourse.mybir as mybir

    f32 = mybir.dt.float32
    nc = tc.nc
    AF = mybir.ActivationFunctionType
    ALU = mybir.AluOpType
    AX = mybir.AxisListType

    # ---------------- pools ----------------
    const = ctx.enter_context(tc.tile_pool(name="const", bufs=1))
    state = ctx.enter_context(tc.tile_pool(name="state", bufs=1))
    xpool = ctx.enter_context(tc.tile_pool(name="xpool", bufs=2))
    wstg = ctx.enter_context(tc.tile_pool(name="wstg", bufs=3))
    zpool = ctx.enter_context(tc.tile_pool(name="zpool", bufs=2))
    small = ctx.enter_context(tc.tile_pool(name="small", bufs=2))
    ps_s = ctx.enter_context(tc.tile_pool(name="ps_s", bufs=1, space="PSUM"))
    ps_y = ctx.enter_context(tc.tile_pool(name="ps_y", bufs=2, space="PSUM"))
    ps_b = ctx.enter_context(tc.tile_pool(name="ps_b", bufs=2, space="PSUM"))

    # ---------------- resident loads ----------------
    wf = const.tile([128, NCH, D, JE], f32)
    for ch in range(NCH):
        nc.sync.dma_start(out=wf[:, ch], in_=wf_d[:, ch])
    inp_i = const.tile([128, NCH, BL, D], f32)
    nc.sync.dma_start(out=inp_i[:], in_=inpi_d[:])
    inp_id = const.tile([128, NG, BL], f32)
    nc.sync.dma_start(out=inp_id[:], in_=inpid_d[:])
    ones = const.tile([128, 8, 128], f32)
    nc.sync.dma_start(out=ones[:], in_=ones_d[:])
    epsb = const.tile([BL, 1], f32)
    nc.vector.memset(epsb[:], EPS)

    # persistent state
    blog = state.tile([128, NCH, J, BL], f32)   # routing logits, [i, (j,b)]
    cbuf = state.tile([128, NCH, J, BL], f32)   # coupling coeffs c
    # s/v in b-partition layout during squash; vbufx rows 0:16 hold v [e,j,b]
    sbT2 = state.tile([BL, J, 32], f32)         # transposed s (+garbage cols)
    vT = state.tile([BL, J, 32], f32)           # v in b-part (+garbage cols)
    vbufx = state.tile([32, J, BL], f32)        # v [e(0:16), j, b] for M1
    vbuf2 = state.tile([E, J, BL], f32)         # v compacted to base-0
    s2T = state.tile([BL, 5, 2, E], f32)
    nrmT = state.tile([BL, 5, 2], f32)
    sclT = state.tile([BL, 5, 2], f32)
    tmpT = state.tile([BL, 5, 2], f32)

    def valid_view(tile_ap):
        """[BL, J, 32] -> strided [BL, 5, 2, 16] view of the valid e-cols.

        Valid cols of j=2q+jj sit at flat offset 64q + 48jj (steps 64/48/1),
        expressed as a step-3 slice over 16-wide chunks.
        """
        return tile_ap.rearrange("b j e -> b (j e)") \
            .rearrange("b (q c s) -> b q c s", q=5, c=4, s=16)[:, :, 0::3, :]

    def squash(iter0):
        """psA/psB diag -> (transpose) -> squash in b-part -> vbufx [e,j,b].

        True s = 0.1*s_raw on iter0: n_true = 0.01*n_raw,
        v = squash_scale(n_true) * 0.1 * s_raw.
        """
        sAP = valid_view(sbT2[:])
        nc.scalar.square(s2T[:], sAP)
        nc.vector.tensor_reduce(nrmT[:], s2T[:], AX.X, ALU.add)
        k = 0.01 if iter0 else 1.0
        # tmpT = 1/(1 + k*n)
        nc.scalar.activation(tmpT[:], nrmT[:], AF.Copy, scale=k)
        nc.vector.tensor_scalar_add(tmpT[:], tmpT[:], 1.0)
        nc.vector.reciprocal(tmpT[:], tmpT[:])
        # sclT = 1/sqrt(k*n + eps)
        nc.scalar.activation(sclT[:], nrmT[:], AF.Sqrt, scale=k, bias=epsb[:])
        nc.vector.reciprocal(sclT[:], sclT[:])
        # sclT = k*n * tmpT * sclT * (0.1 iter0)
        nc.vector.tensor_mul(sclT[:], sclT[:], tmpT[:])
        kk = k * (0.1 if iter0 else 1.0)
        nc.scalar.activation(sclT[:], sclT[:], AF.Copy, scale=kk)
        nc.vector.tensor_mul(sclT[:], sclT[:], nrmT[:])
        # vT = s * scale (broadcast over e), on the valid cols view
        nc.vector.tensor_tensor(
            valid_view(vT[:]),
            sAP,
            sclT[:].unsqueeze(3).broadcast_to([BL, 5, 2, 16]),
            ALU.mult)
        # transpose back: valid v of j lands at vbufx rows 16*(j%2)+0:16;
        # compact to base-0 via SBUF->SBUF DMA (engines are lane-locked,
        # DMA is address-based so it can shift partitions)
        for j in range(J):
            nc.vector.transpose(vbufx[:, j], vT[:, j])
        for j in range(J):
            r = 16 * (j % 2)
            nc.sync.dma_start(out=vbuf2[:, j], in_=vbufx[r:r + 16, j])

    def v_ap(j):
        """M1/moving view of v for capsule j: [16, BL]."""
        return vbuf2[:, j]

    def s_step(it):
        """cbuf (or uniform 0.1 if it==0) -> s matmuls -> sbuf_s [E,J,BL].

        Per (ch, d, j): psS[e, j, b] += wf[i,(j,e)]^T X[i, (j,b)],
        PSUM-accumulated over the 128 (ch,d) pairs.
        """
        psA = ps_s.tile([128, 8, BL], f32, name=f"psA{it}", tag="psA")
        psB = ps_s.tile([32, 2, BL], f32, name=f"psB{it}", tag="psB")
        nmm = NCH * D
        k = 0
        for ch in range(NCH):
            X = xpool.tile([128, D, J, BL], f32, name=f"X{it}_{ch}", tag="X")
            if it == 0:
                src = inp_i[:, ch].rearrange("p b d -> p d b") \
                    .unsqueeze(2).broadcast_to([128, D, J, BL])
                nc.gpsimd.tensor_scalar_mul(X[:], src, 1.0)
            else:
                cin = cbuf[:, ch].unsqueeze(1).broadcast_to([128, D, J, BL])
                iin = inp_i[:, ch].rearrange("p b d -> p d b") \
                    .unsqueeze(2).broadcast_to([128, D, J, BL])
                nc.gpsimd.tensor_tensor(X[:], cin, iin, ALU.mult)
            dbg = _kernel_body.debug_tensors
            if it == 0 and ch == 0 and "x0" in dbg:
                nc.sync.dma_start(out=dbg["x0"][:], in_=X[:])
            for d in range(D):
                st = (k == 0)
                sp = (k == nmm - 1)
                nc.tensor.matmul(
                    psA[:].rearrange("p j b -> p (j b)"),
                    wf[:, ch, d, 0:128],
                    X[:, d, 0:8].rearrange("p j b -> p (j b)"),
                    start=st, stop=sp)
                nc.tensor.matmul(
                    psB[:].rearrange("p j b -> p (j b)"),
                    wf[:, ch, d, 128:160],
                    X[:, d, 8:10].rearrange("p j b -> p (j b)"),
                    start=st, stop=sp)
                k += 1
        if it == 0 and "psa0" in _kernel_body.debug_tensors:
            dbg = _kernel_body.debug_tensors
            pacopy = small.tile([128, 8, BL], f32, name="pacopy", tag="pac")
            nc.vector.tensor_copy(pacopy[:], psA[:])
            nc.sync.dma_start(out=dbg["psa0"][:], in_=pacopy[:])
            pbcopy = small.tile([32, 2, BL], f32, name="pbcopy", tag="pbc")
            nc.vector.tensor_copy(pbcopy[:], psB[:])
            nc.sync.dma_start(out=dbg["psb0"][:], in_=pbcopy[:])
        # diagonal extract via 32x32 DVE transposes (PSUM compute reads must
        # be 32-partition aligned; each transpose grabs a j-pair's rows and
        # lands s[b, e] in b-partition layout, valid cols at 16*(j%2))
        for q in range(4):
            for jj in range(2):
                j = 2 * q + jj
                nc.vector.transpose(sbT2[:, j], psA[32 * q:32 * (q + 1), j])
        nc.vector.transpose(sbT2[:, 8], psB[:, 0])
        nc.vector.transpose(sbT2[:, 9], psB[:, 1])

    def t_step(it):
        """vbuf -> blog (it==0: overwrite; else accumulate)."""
        for sup in range(NCH):
            bp = ps_b.tile([128, J, BL], f32, name=f"bp{it}_{sup}", tag="bp")
            for gq in range(8):
                g = sup * 8 + gq
                stg = wstg.tile([E, J, 128], f32, name=f"wst{it}_{g}", tag="wst")
                nc.sync.dma_start(out=stg[:], in_=wm1_d[:, :, 128 * g:128 * (g + 1)])
                yp = ps_y.tile([128, J, BL], f32, name=f"yp{it}_{g}", tag="yp")
                for j in range(J):
                    nc.tensor.matmul(yp[:, j], stg[:, j], v_ap(j))
                Z = zpool.tile([128, J, BL], f32, name=f"Z{it}_{g}", tag="Z")
                nc.vector.tensor_tensor(
                    Z[:], yp[:],
                    inp_id[:, g].unsqueeze(1).broadcast_to([128, J, BL]),
                    ALU.mult)
                nc.tensor.matmul(bp[:].rearrange("p j b -> p (j b)"),
                                 ones[:, gq],
                                 Z[:].rearrange("p j b -> p (j b)"),
                                 start=(gq == 0), stop=(gq == 7))
            if it == 0:
                nc.scalar.copy(blog[:, sup], bp[:])
            else:
                nc.vector.tensor_add(blog[:, sup], blog[:, sup], bp[:])

    def softmax():
        """cbuf = softmax_j(blog) (no max-subtraction; logits are small)."""
        for ch in range(NCH):
            nc.scalar.activation(cbuf[:, ch], blog[:, ch], AF.Exp)
            ssum = small.tile([128, BL], f32, name=f"ss{ch}", tag="ssum")
            nc.vector.tensor_reduce(
                ssum[:], cbuf[:, ch].rearrange("p j b -> p b j"),
                AX.X, ALU.add)
            nc.vector.reciprocal(ssum[:], ssum[:])
            nc.vector.tensor_mul(
                cbuf[:, ch], cbuf[:, ch],
                ssum[:].unsqueeze(1).broadcast_to([128, J, BL]))

    # ---------------- the routing schedule ----------------
    dbg = _kernel_body.debug_tensors
    s_step(0)
    if "s0" in dbg:
        nc.sync.dma_start(out=dbg["s0"][:], in_=sbT2[:])
    squash(True)          # v0
    if "v0" in dbg:
        nc.sync.dma_start(out=dbg["v0"][:], in_=vbuf2[:])
    t_step(0)             # blog = t0
    if "blog0" in dbg:
        nc.sync.dma_start(out=dbg["blog0"][:], in_=blog[:])
    softmax()             # c1
    if "c1" in dbg:
        nc.sync.dma_start(out=dbg["c1"][:], in_=cbuf[:])
    s_step(1)
    squash(False)         # v1
    t_step(1)             # blog += t1
    softmax()             # c2
    s_step(2)
    squash(False)         # v2

    # ---------------- output ----------------
    # out[b, j, 0:16] = v2[e, j, b]
    for j in range(J):
        nc.sync.dma_start(out=out_d[:, j, 0:E].rearrange("b e -> e b"),
                          in_=v_ap(j))
    # out[b, j, 16:2064] = c2[b, j, i], i = ch*128 + p
    for j in range(J):
        for b in range(BL):
            nc.sync.dma_start(
                out=out_d[b, j, E:OUTW].rearrange("(c p) -> p c", p=128),
                in_=cbuf[:, :, j, b])


def kernel(inputs, W):
    global _PROGRAM
    from concourse.bass_utils import run_bass_kernel_spmd

    shared, per_core = _host_prep(np.asarray(inputs), np.asarray(W))
    if _PROGRAM is None:
        _PROGRAM = _build_program()
    in_maps = [{**shared, **pc} for pc in per_core]
    res = run_bass_kernel_spmd(_PROGRAM, in_maps, core_ids=list(range(NCORES)))
    out = np.concatenate([r["out"] for r in res.results], axis=0)
    return out.astype(np.float32)


if __name__ == "__main__":
    rng = np.random.default_rng(0)
    x = rng.standard_normal((B, I, D), dtype=np.float32)
    w = rng.standard_normal((J, I, E, D), dtype=np.float32)
    y = kernel(x, w)
    print(y.shape, y.dtype)



# revision 6
# speedup vs baseline: 10.9063x; 10.9063x over previous
"""CapsuleLayer dynamic-routing kernel for Trainium2 (8 NeuronCores), v2.

Problem: B=256, I=2048, D=8 input capsules -> J=10, E=16 output capsules,
3 routing iterations.  Output = concat([v2, c2], axis=-1) -> [B, J, E+I].

Sharding: pure data parallelism over batch (32 batches/core), W replicated.

v2 design (vs v1): bf16 matmul datapath, all weights SBUF-resident (both
layouts), ~24 large DMAs total, block-diagonal vblk for the t-step M1
(K=128 instead of 2560 K=16 matmuls), X=c*inputs on DVE, Y-copy on ACT,
chunk-pipelined t->softmax->X->s schedule.

Per-core steps (u_hat never materialized):
  s-step:  s[b,j,e] = sum_{i,d} X[b,j,i,d] W[j,i,e,d],  X = c (.) inputs
           diag trick: psA[(j8,e),(j8,b)] += wf[i,(j,e)]^T X[i,(j,b)]
           per (ch,d), PSUM-accumulated; psB for j=8,9.  it0: X==inputs
           (c uniform, 0.1 folded into squash), no diag needed: N=32.
  t-step:  M1: Y[(i,d)chunk,(j,b)] = wm1chunk[(j,e),(i,d)]^T vblk[(j,e),(j,b)]
           (vblk block-diagonal, built directly by the squash transposes)
           Z = Y (.) inputs (ACT copy PSUM->SBUF bf16, DVE multiply)
           M2: blog[i,(j,b)] += ones_blkdiag^T Z  (sums d)
  softmax over j without max-subtraction (logits are O(few)), per-sup
  pipelined right after its blog rows are produced.

Layouts (i = ch*128 + p; k = i*8 + d, g = k/128, q = k%128):
  wf    [128,16,8,160] bf16  wf[p,ch,d,16j+e] = W[j, 128ch+p, e, d]
  wm1a  [128,128,128]  bf16  wm1a[16j+e,g,c]  = W[j, (128g+c)/8, e, (128g+c)%8], j<8
  wm1b  [32,128,128]   bf16  same, j in {8,9}, row 16(j-8)+e
  ones  [128,8,128]    bf16  ones[q,gq,m] = (m == 16*gq + q//8)
  inp_i [128,16,8,32]  bf16  inp_i[p,ch,d,b] = x[b0+b, 128ch+p, d]
  inp_id[128,128,32]   bf16  inp_id[q,g,b]   = x[b0+b, 16g+q//8, q%8]
"""

import numpy as np

B, I, D, J, E = 256, 2048, 8, 10, 16
NCORES = 8
BL = B // NCORES          # 32 batches per core
NCH = I // 128            # 16 i-chunks of 128
NG = (I * D) // 128       # 128 (i,d)-groups of 128
JE = J * E                # 160
JB = J * BL               # 320
OUTW = E + I              # 2064
EPS = 1e-7
CB = 2                    # (i,d)-chunks per Y/Z batch in the t-step

_PROGRAM = None
_BUILD_DEBUG = False


def _host_prep(inputs, W):
    """Build all DRAM-side arrays. Returns (shared dict, per-core list)."""
    import concourse.mybir as mybir
    bf16 = mybir.dt.np(mybir.dt.bfloat16)
    W = np.ascontiguousarray(W, dtype=np.float32)
    inputs = np.ascontiguousarray(inputs, dtype=np.float32)

    # wf[p, ch, d, 16j+e] = W[j, ch*128+p, e, d]
    wf = W.transpose(1, 3, 0, 2).reshape(NCH, 128, D, JE)
    wf = np.ascontiguousarray(wf.transpose(1, 0, 2, 3)).astype(bf16)

    # wm1[16j+e, (i,d)] = W[j, i, e, d], split j<8 / j>=8, grouped by 128
    wm1 = W.transpose(0, 2, 1, 3).reshape(J, E, I * D)
    wm1a = np.ascontiguousarray(
        wm1[0:8].reshape(128, NG, 128)).astype(bf16)
    wm1b = np.ascontiguousarray(
        wm1[8:10].reshape(32, NG, 128)).astype(bf16)

    # ones[q, gq, m] = 1 iff m == 16*gq + q//8
    ones = np.zeros((128, 8, 128), dtype=np.float32)
    q = np.arange(128)
    for gq in range(8):
        ones[q, gq, 16 * gq + q // 8] = 1.0
    ones = ones.astype(bf16)

    shared = {"wf": wf, "wm1a": wm1a, "wm1b": wm1b, "ones": ones}

    per_core = []
    for m in range(NCORES):
        sl = inputs[m * BL:(m + 1) * BL]                  # [32, 2048, 8]
        # inp_i[p, ch, d, b] = sl[b, ch*128+p, d]
        inp_i = np.ascontiguousarray(
            sl.reshape(BL, NCH, 128, D).transpose(2, 1, 3, 0)).astype(bf16)
        # inp_id[q, g, b] = sl[b, g*16 + q//8, q%8]
        inp_id = np.ascontiguousarray(
            sl.reshape(BL, NG, 16, 8).transpose(2, 3, 1, 0)
            .reshape(128, NG, BL)).astype(bf16)
        per_core.append({"inp_i": inp_i, "inp_id": inp_id})
    return shared, per_core


def _build_program():
    from contextlib import ExitStack
    import concourse.mybir as mybir
    from concourse import bacc
    from concourse.tile import TileContext

    f32 = mybir.dt.float32
    bf16 = mybir.dt.bfloat16
    nc = bacc.Bacc()

    wf_d = nc.dram_tensor("wf", [128, NCH, D, JE], bf16, kind="ExternalInput")
    wm1a_d = nc.dram_tensor("wm1a", [128, NG, 128], bf16, kind="ExternalInput")
    wm1b_d = nc.dram_tensor("wm1b", [32, NG, 128], bf16, kind="ExternalInput")
    ones_d = nc.dram_tensor("ones", [128, 8, 128], bf16, kind="ExternalInput")
    inpi_d = nc.dram_tensor("inp_i", [128, NCH, D, BL], bf16,
                            kind="ExternalInput")
    inpid_d = nc.dram_tensor("inp_id", [128, NG, BL], bf16,
                             kind="ExternalInput")
    outv_d = nc.dram_tensor("outv", [BL, J, E], f32, kind="ExternalOutput")
    outc_d = nc.dram_tensor("outc", [128, NCH, J, BL], bf16,
                            kind="ExternalOutput")

    _kernel_body.debug_tensors = {}
    if _BUILD_DEBUG:
        _kernel_body.debug_tensors = {
            "sbT2_0": nc.dram_tensor("dbg_sbT2_0", [BL, J, 32], f32,
                                     kind="ExternalOutput"),
            "vblkA_0": nc.dram_tensor("dbg_vblkA_0", [128, 8, BL], bf16,
                                      kind="ExternalOutput"),
            "vblkB_0": nc.dram_tensor("dbg_vblkB_0", [32, 2, BL], bf16,
                                      kind="ExternalOutput"),
            "blog_0": nc.dram_tensor("dbg_blog_0", [128, NCH, J, BL], f32,
                                     kind="ExternalOutput"),
            "cbuf_0": nc.dram_tensor("dbg_cbuf_0", [128, NCH, J, BL], bf16,
                                     kind="ExternalOutput"),
            "sbT2_1": nc.dram_tensor("dbg_sbT2_1", [BL, J, 32], f32,
                                     kind="ExternalOutput"),
        }

    with ExitStack() as ctx:
        tc = ctx.enter_context(TileContext(nc))
        _kernel_body(ctx, tc, wf_d, wm1a_d, wm1b_d, ones_d, inpi_d, inpid_d,
                     outv_d, outc_d)
    nc.compile()
    return nc


def _kernel_body(ctx, tc, wf_d, wm1a_d, wm1b_d, ones_d, inpi_d, inpid_d,
                 outv_d, outc_d):
    import concourse.mybir as mybir

    f32 = mybir.dt.float32
    bf16 = mybir.dt.bfloat16
    nc = tc.nc
    AF = mybir.ActivationFunctionType
    ALU = mybir.AluOpType
    AX = mybir.AxisListType

    # ---------------- pools ----------------
    const = ctx.enter_context(tc.tile_pool(name="const", bufs=1))
    state = ctx.enter_context(tc.tile_pool(name="state", bufs=1))
    xpool = ctx.enter_context(tc.tile_pool(name="xpool", bufs=2))
    ypool = ctx.enter_context(tc.tile_pool(name="ypool", bufs=2))
    zpool = ctx.enter_context(tc.tile_pool(name="zpool", bufs=2))
    small = ctx.enter_context(tc.tile_pool(name="small", bufs=2))
    ps_s = ctx.enter_context(tc.tile_pool(name="ps_s", bufs=1, space="PSUM"))
    ps_y = ctx.enter_context(tc.tile_pool(name="ps_y", bufs=2, space="PSUM"))
    ps_b = ctx.enter_context(tc.tile_pool(name="ps_b", bufs=2, space="PSUM"))

    # ---------------- resident loads ----------------
    wf = const.tile([128, NCH, D, JE], bf16)
    for ch in range(NCH):
        nc.sync.dma_start(out=wf[:, ch], in_=wf_d[:, ch])
    inp_i = const.tile([128, NCH, D, BL], bf16)
    nc.sync.dma_start(out=inp_i[:], in_=inpi_d[:])
    inp_id = const.tile([128, NG, BL], bf16)
    nc.sync.dma_start(out=inp_id[:], in_=inpid_d[:])
    wm1a = const.tile([128, NG, 128], bf16)
    for h in range(4):
        nc.scalar.dma_start(out=wm1a[:, 32 * h:32 * (h + 1)],
                            in_=wm1a_d[:, 32 * h:32 * (h + 1)])
    wm1b = const.tile([32, NG, 128], bf16)
    nc.scalar.dma_start(out=wm1b[:], in_=wm1b_d[:])
    ones = const.tile([128, 8, 128], bf16)
    nc.scalar.dma_start(out=ones[:], in_=ones_d[:])
    epsb = const.tile([BL, 1], f32)
    nc.vector.memset(epsb[:], EPS)

    # persistent state
    blog = state.tile([128, NCH, J, BL], f32)   # routing logits, [i, (j,b)]
    cbuf = state.tile([128, NCH, J, BL], bf16)  # coupling coeffs c
    sbT2 = state.tile([BL, J, 32], f32)         # transposed s (+garbage cols)
    vT = state.tile([BL, J, 32], bf16)          # v in b-part (garbage cols=0)
    vblkA = state.tile([128, 8, BL], bf16)      # block-diag v, rows (j8,e)
    vblkB = state.tile([32, 2, BL], bf16)       # block-diag v, rows (j2,e)
    vout = state.tile([BL, J, E], f32)          # final v for output
    s2T = state.tile([BL, 5, 2, E], f32)
    nrmT = state.tile([BL, 5, 2], f32)
    sclT = state.tile([BL, 5, 2], f32)
    tmpT = state.tile([BL, 5, 2], f32)
    nc.vector.memset(vT[:], 0.0)
    nc.vector.memset(vblkA[:], 0.0)
    nc.vector.memset(vblkB[:], 0.0)

    def valid_view(tile_ap):
        """[BL, J, 32] -> strided [BL, 5, 2, 16] view of the valid e-cols.

        Valid cols of j=2q+jj sit at flat offset 64q + 48jj (steps 64/48/1),
        expressed as a step-3 slice over 16-wide chunks.
        """
        return tile_ap.rearrange("b j e -> b (j e)") \
            .rearrange("b (q c s) -> b q c s", q=5, c=4, s=16)[:, :, 0::3, :]

    def squash(iter0, final=False):
        """psA/psB diag -> (transpose) -> squash in b-part -> vblkA/vblkB.

        True s = 0.1*s_raw on iter0: n_true = 0.01*n_raw,
        v = squash_scale(n_true) * 0.1 * s_raw.
        """
        sAP = valid_view(sbT2[:])
        nc.scalar.square(s2T[:], sAP)
        nc.vector.tensor_reduce(nrmT[:], s2T[:], AX.X, ALU.add)
        k = 0.01 if iter0 else 1.0
        # tmpT = 1/(1 + k*n)
        nc.scalar.activation(tmpT[:], nrmT[:], AF.Copy, scale=k)
        nc.vector.tensor_scalar_add(tmpT[:], tmpT[:], 1.0)
        nc.vector.reciprocal(tmpT[:], tmpT[:])
        # sclT = 1/sqrt(k*n + eps)
        nc.scalar.activation(sclT[:], nrmT[:], AF.Sqrt, scale=k, bias=epsb[:])
        nc.vector.reciprocal(sclT[:], sclT[:])
        # sclT = k*n * tmpT * sclT * (0.1 iter0)
        nc.vector.tensor_mul(sclT[:], sclT[:], tmpT[:])
        kk = k * (0.1 if iter0 else 1.0)
        nc.scalar.activation(sclT[:], sclT[:], AF.Copy, scale=kk)
        nc.vector.tensor_mul(sclT[:], sclT[:], nrmT[:])
        scl_b = sclT[:].unsqueeze(3).broadcast_to([BL, 5, 2, 16])
        # vT = s * scale (broadcast over e), on the valid cols view only;
        # garbage cols of vT stay 0 so the back-transposes below write
        # exact block-diagonal vblk tiles.
        nc.vector.tensor_tensor(valid_view(vT[:]), sAP, scl_b, ALU.mult)
        if final:
            nc.vector.tensor_tensor(
                vout[:].rearrange("b (q c) e -> b q c e", q=5, c=2),
                sAP, scl_b, ALU.mult)
        # back-transposes: vT[:, j] is [32b x 32] with v_j at cols
        # 16*(j%2)+e, zeros elsewhere; its transpose is the [32, 32]
        # block-diagonal tile of vblk at rows 32*(j//2), col-block j.
        for j in range(8):
            q = j // 2
            nc.vector.transpose(
                vblkA[32 * q:32 * (q + 1), j], vT[:, j])
        nc.vector.transpose(vblkB[:, 0], vT[:, 8])
        nc.vector.transpose(vblkB[:, 1], vT[:, 9])

    def s_step(it):
        """cbuf (or uniform 0.1 if it==0) -> s matmuls -> psA/psB diag ->
        forward transposes into sbT2 (b-partition layout for squash)."""
        if it == 0:
            psA = ps_s.tile([128, 8, BL], f32, name="psA0", tag="psA")
            psB = ps_s.tile([32, 2, BL], f32, name="psB0", tag="psB")
            k = 0
            for ch in range(NCH):
                for d in range(D):
                    st, sp = (k == 0), (k == NCH * D - 1)
                    rhs = inp_i[:, ch, d]
                    nc.tensor.matmul(psA[:, 0], wf[:, ch, d, 0:128], rhs,
                                     start=st, stop=sp)
                    nc.tensor.matmul(psB[:, 0], wf[:, ch, d, 128:160], rhs,
                                     start=st, stop=sp)
                    k += 1
            for j in range(8):
                q = j // 2
                nc.vector.transpose(sbT2[:, j], psA[32 * q:32 * (q + 1), 0])
            nc.vector.transpose(sbT2[:, 8], psB[:, 0])
            nc.vector.transpose(sbT2[:, 9], psB[:, 0])
        else:
            psA = ps_s.tile([128, 8, BL], f32, name=f"psA{it}", tag="psA")
            psB = ps_s.tile([32, 2, BL], f32, name=f"psB{it}", tag="psB")
            k = 0
            for ch in range(NCH):
                X = xpool.tile([128, D, J, BL], bf16, name=f"X{it}_{ch}",
                               tag="X")
                cin = cbuf[:, ch].unsqueeze(1).broadcast_to([128, D, J, BL])
                iin = inp_i[:, ch].unsqueeze(2).broadcast_to([128, D, J, BL])
                nc.vector.tensor_tensor(X[:], cin, iin, ALU.mult)
                for d in range(D):
                    st, sp = (k == 0), (k == NCH * D - 1)
                    nc.tensor.matmul(
                        psA[:].rearrange("p j b -> p (j b)"),
                        wf[:, ch, d, 0:128],
                        X[:, d, 0:8].rearrange("p j b -> p (j b)"),
                        start=st, stop=sp)
                    nc.tensor.matmul(
                        psB[:].rearrange("p j b -> p (j b)"),
                        wf[:, ch, d, 128:160],
                        X[:, d, 8:10].rearrange("p j b -> p (j b)"),
                        start=st, stop=sp)
                    k += 1
            for j in range(8):
                q = j // 2
                nc.vector.transpose(sbT2[:, j],
                                    psA[32 * q:32 * (q + 1), j])
            nc.vector.transpose(sbT2[:, 8], psB[:, 0])
            nc.vector.transpose(sbT2[:, 9], psB[:, 1])

    def t_step(it):
        """vblk -> blog (it==0: overwrite; else accumulate) + softmax,
        pipelined per sup (sup == ch of blog/cbuf)."""
        for sup in range(NCH):
            bp = ps_b.tile([128, J, BL], f32, name=f"bp{it}_{sup}", tag="bp")
            for half in range(8 // CB):
                # 512 f32 per cc: pad to a full PSUM bank so no matmul
                # output crosses a bank boundary (writes past a bank edge
                # corrupt silently).
                yp = ps_y.tile([128, CB, 512], f32,
                               name=f"yp{it}_{sup}_{half}", tag="yp")
                for cc in range(CB):
                    g = sup * 8 + half * CB + cc
                    nc.tensor.matmul(
                        yp[:, cc, 0:256],
                        wm1a[:, g], vblkA[:].rearrange("p j b -> p (j b)"),
                        start=True, stop=True)
                    nc.tensor.matmul(
                        yp[:, cc, 256:320],
                        wm1b[:, g], vblkB[:].rearrange("p j b -> p (j b)"),
                        start=True, stop=True)
                ysb = ypool.tile([128, CB, J, BL], bf16,
                                 name=f"ysb{it}_{sup}_{half}", tag="ysb")
                nc.scalar.copy(
                    ysb[:].rearrange("p c j b -> p c (j b)"),
                    yp[:, :, 0:JB])
                Z = zpool.tile([128, CB, J, BL], bf16,
                               name=f"Z{it}_{sup}_{half}", tag="Z")
                g0 = sup * 8 + half * CB
                nc.vector.tensor_tensor(
                    Z[:], ysb[:],
                    inp_id[:, g0:g0 + CB].unsqueeze(2)
                    .broadcast_to([128, CB, J, BL]),
                    ALU.mult)
                for cc in range(CB):
                    gq = half * CB + cc
                    nc.tensor.matmul(
                        bp[:].rearrange("p j b -> p (j b)"),
                        ones[:, gq],
                        Z[:, cc].rearrange("p j b -> p (j b)"),
                        start=(gq == 0), stop=(gq == 7))
            if it == 0:
                nc.scalar.copy(blog[:, sup], bp[:])
            else:
                nc.vector.tensor_add(blog[:, sup], blog[:, sup], bp[:])
            # softmax over j for this sup (logits are small; no max-sub)
            nc.scalar.activation(cbuf[:, sup], blog[:, sup], AF.Exp)
            ssum = small.tile([128, BL], f32, name=f"ss{it}_{sup}",
                              tag="ssum")
            nc.vector.tensor_reduce(
                ssum[:], cbuf[:, sup].rearrange("p j b -> p b j"),
                AX.X, ALU.add)
            nc.vector.reciprocal(ssum[:], ssum[:])
            nc.vector.tensor_mul(
                cbuf[:, sup], cbuf[:, sup],
                ssum[:].unsqueeze(1).broadcast_to([128, J, BL]))

    # ---------------- the routing schedule ----------------
    dbg = _kernel_body.debug_tensors
    s_step(0)
    if "sbT2_0" in dbg:
        nc.sync.dma_start(out=dbg["sbT2_0"][:], in_=sbT2[:])
    squash(True)          # v0
    if "vblkA_0" in dbg:
        nc.sync.dma_start(out=dbg["vblkA_0"][:], in_=vblkA[:])
        nc.sync.dma_start(out=dbg["vblkB_0"][:], in_=vblkB[:])
    t_step(0)             # blog = t0, c1 = softmax(blog)
    if "blog_0" in dbg:
        nc.sync.dma_start(out=dbg["blog_0"][:], in_=blog[:])
        nc.sync.dma_start(out=dbg["cbuf_0"][:], in_=cbuf[:])
    s_step(1)
    if "sbT2_1" in dbg:
        nc.sync.dma_start(out=dbg["sbT2_1"][:], in_=sbT2[:])
    squash(False)         # v1
    t_step(1)             # blog += t1, c2 = softmax(blog)
    s_step(2)
    squash(False, final=True)  # v2 -> vout

    # ---------------- output ----------------
    nc.sync.dma_start(out=outv_d[:], in_=vout[:])
    nc.sync.dma_start(out=outc_d[:], in_=cbuf[:])


def kernel(inputs, W):
    global _PROGRAM
    from concourse.bass_utils import run_bass_kernel_spmd

    shared, per_core = _host_prep(np.asarray(inputs), np.asarray(W))
    if _PROGRAM is None:
        _PROGRAM = _build_program()
    in_maps = [{**shared, **pc} for pc in per_core]
    res = run_bass_kernel_spmd(_PROGRAM, in_maps, core_ids=list(range(NCORES)))
    outs = []
    for r in res.results:
        v = np.asarray(r["outv"], dtype=np.float32)        # [BL, J, E]
        c = np.asarray(r["outc"]).astype(np.float32)       # [128, NCH, J, BL]
        c = c.transpose(3, 2, 1, 0).reshape(BL, J, I)
        outs.append(np.concatenate([v, c], axis=-1))
    return np.concatenate(outs, axis=0).astype(np.float32)


if __name__ == "__main__":
    rng = np.random.default_rng(0)
    x = rng.standard_normal((B, I, D), dtype=np.float32)
    w = rng.standard_normal((J, I, E, D), dtype=np.float32)
    y = kernel(x, w)
    print(y.shape, y.dtype)


# revision 9
# speedup vs baseline: 11.6699x; 1.0700x over previous
"""CapsuleLayer dynamic-routing kernel for Trainium2 (8 NeuronCores), v2.

Problem: B=256, I=2048, D=8 input capsules -> J=10, E=16 output capsules,
3 routing iterations.  Output = concat([v2, c2], axis=-1) -> [B, J, E+I].

Sharding: pure data parallelism over batch (32 batches/core), W replicated.

v2 design (vs v1): bf16 matmul datapath, all weights SBUF-resident (both
layouts), ~24 large DMAs total, block-diagonal vblk for the t-step M1
(K=128 instead of 2560 K=16 matmuls), X=c*inputs on DVE, Y-copy on ACT,
chunk-pipelined t->softmax->X->s schedule.

Per-core steps (u_hat never materialized):
  s-step:  s[b,j,e] = sum_{i,d} X[b,j,i,d] W[j,i,e,d],  X = c (.) inputs
           diag trick: psA[(j8,e),(j8,b)] += wf[i,(j,e)]^T X[i,(j,b)]
           per (ch,d), PSUM-accumulated; psB for j=8,9.  it0: X==inputs
           (c uniform, 0.1 folded into squash), no diag needed: N=32.
  t-step:  M1: Y[(i,d)chunk,(j,b)] = wm1chunk[(j,e),(i,d)]^T vblk[(j,e),(j,b)]
           (vblk block-diagonal, built directly by the squash transposes)
           Z = Y (.) inputs (ACT copy PSUM->SBUF bf16, DVE multiply)
           M2: blog[i,(j,b)] += ones_blkdiag^T Z  (sums d)
  softmax over j without max-subtraction (logits are O(few)), per-sup
  pipelined right after its blog rows are produced.

Layouts (i = ch*128 + p; k = i*8 + d, g = k/128, q = k%128):
  wf    [128,16,8,160] bf16  wf[p,ch,d,16j+e] = W[j, 128ch+p, e, d]
  wm1a  [128,128,128]  bf16  wm1a[16j+e,g,c]  = W[j, (128g+c)/8, e, (128g+c)%8], j<8
  wm1b  [32,128,128]   bf16  same, j in {8,9}, row 16(j-8)+e
  ones  [128,8,128]    bf16  ones[q,gq,m] = (m == 16*gq + q//8)
  inp_i [128,16,8,32]  bf16  inp_i[p,ch,d,b] = x[b0+b, 128ch+p, d]
  inp_id[128,128,32]   bf16  inp_id[q,g,b]   = x[b0+b, 16g+q//8, q%8]
"""

import numpy as np

B, I, D, J, E = 256, 2048, 8, 10, 16
NCORES = 8
BL = B // NCORES          # 32 batches per core
NCH = I // 128            # 16 i-chunks of 128
NG = (I * D) // 128       # 128 (i,d)-groups of 128
JE = J * E                # 160
JB = J * BL               # 320
OUTW = E + I              # 2064
EPS = 1e-7
CB = 2                    # (i,d)-chunks per Y/Z batch in the t-step

_PROGRAM = None
_BUILD_DEBUG = False


def _host_prep(inputs, W):
    """Build all DRAM-side arrays. Returns (shared dict, per-core list)."""
    import concourse.mybir as mybir
    bf16 = mybir.dt.np(mybir.dt.float16)
    W = np.ascontiguousarray(W, dtype=np.float32)
    inputs = np.ascontiguousarray(inputs, dtype=np.float32)

    # wf[p, ch, d, 16j+e] = W[j, ch*128+p, e, d]
    wf = W.transpose(1, 3, 0, 2).reshape(NCH, 128, D, JE)
    wf = np.ascontiguousarray(wf.transpose(1, 0, 2, 3)).astype(bf16)

    # wm1[16j+e, (i,d)] = W[j, i, e, d], split j<8 / j>=8, grouped by 128
    wm1 = W.transpose(0, 2, 1, 3).reshape(J, E, I * D)
    wm1a = np.ascontiguousarray(
        wm1[0:8].reshape(128, NG, 128)).astype(bf16)
    wm1b = np.ascontiguousarray(
        wm1[8:10].reshape(32, NG, 128)).astype(bf16)

    # ones32[q, h, m] = 1 iff m == 16*h + q//8  (h = chunk parity)
    ones = np.zeros((128, 2, 32), dtype=np.float32)
    q = np.arange(128)
    for h in range(2):
        ones[q, h, 16 * h + q // 8] = 1.0
    ones = ones.astype(bf16)

    shared = {"wf": wf, "wm1a": wm1a, "wm1b": wm1b, "ones": ones}

    per_core = []
    for m in range(NCORES):
        sl = inputs[m * BL:(m + 1) * BL]                  # [32, 2048, 8]
        # inp_i[p, ch, d, b] = sl[b, ch*128+p, d]
        inp_i = np.ascontiguousarray(
            sl.reshape(BL, NCH, 128, D).transpose(2, 1, 3, 0)).astype(bf16)
        # inp_id[q, g, b] = sl[b, g*16 + q//8, q%8]
        inp_id = np.ascontiguousarray(
            sl.reshape(BL, NG, 16, 8).transpose(2, 3, 1, 0)
            .reshape(128, NG, BL)).astype(bf16)
        per_core.append({"inp_i": inp_i, "inp_id": inp_id})
    return shared, per_core


def _build_program():
    from contextlib import ExitStack
    import concourse.mybir as mybir
    from concourse import bacc
    from concourse.tile import TileContext

    f32 = mybir.dt.float32
    bf16 = mybir.dt.float16
    nc = bacc.Bacc()

    wf_d = nc.dram_tensor("wf", [128, NCH, D, JE], bf16, kind="ExternalInput")
    wm1a_d = nc.dram_tensor("wm1a", [128, NG, 128], bf16, kind="ExternalInput")
    wm1b_d = nc.dram_tensor("wm1b", [32, NG, 128], bf16, kind="ExternalInput")
    ones_d = nc.dram_tensor("ones", [128, 2, 32], bf16, kind="ExternalInput")
    inpi_d = nc.dram_tensor("inp_i", [128, NCH, D, BL], bf16,
                            kind="ExternalInput")
    inpid_d = nc.dram_tensor("inp_id", [128, NG, BL], bf16,
                             kind="ExternalInput")
    outv_d = nc.dram_tensor("outv", [BL, J, E], f32, kind="ExternalOutput")
    outc_d = nc.dram_tensor("outc", [128, NCH, J, BL], mybir.dt.bfloat16,
                            kind="ExternalOutput")

    _kernel_body.debug_tensors = {}
    if _BUILD_DEBUG:
        _kernel_body.debug_tensors = {
            "sbT2_0": nc.dram_tensor("dbg_sbT2_0", [BL, J, 32], f32,
                                     kind="ExternalOutput"),
            "vblkA_0": nc.dram_tensor("dbg_vblkA_0", [128, 8, BL], bf16,
                                      kind="ExternalOutput"),
            "vblkB_0": nc.dram_tensor("dbg_vblkB_0", [32, 2, BL], bf16,
                                      kind="ExternalOutput"),
            "blog_0": nc.dram_tensor("dbg_blog_0", [128, NCH, J, BL], f32,
                                     kind="ExternalOutput"),
            "cbuf_0": nc.dram_tensor("dbg_cbuf_0", [128, NCH, J, BL], mybir.dt.bfloat16,
                                     kind="ExternalOutput"),
            "sbT2_1": nc.dram_tensor("dbg_sbT2_1", [BL, J, 32], f32,
                                     kind="ExternalOutput"),
        }

    with ExitStack() as ctx:
        tc = ctx.enter_context(TileContext(nc))
        _kernel_body(ctx, tc, wf_d, wm1a_d, wm1b_d, ones_d, inpi_d, inpid_d,
                     outv_d, outc_d)
    nc.compile()
    return nc


def _kernel_body(ctx, tc, wf_d, wm1a_d, wm1b_d, ones_d, inpi_d, inpid_d,
                 outv_d, outc_d):
    import concourse.mybir as mybir

    f32 = mybir.dt.float32
    bf16 = mybir.dt.float16
    b16r = mybir.dt.bfloat16
    nc = tc.nc
    AF = mybir.ActivationFunctionType
    ALU = mybir.AluOpType
    AX = mybir.AxisListType

    # ---------------- pools ----------------
    const = ctx.enter_context(tc.tile_pool(name="const", bufs=1))
    state = ctx.enter_context(tc.tile_pool(name="state", bufs=1))
    xpool = ctx.enter_context(tc.tile_pool(name="xpool", bufs=2))
    ypool = ctx.enter_context(tc.tile_pool(name="ypool", bufs=2))
    zpool = ctx.enter_context(tc.tile_pool(name="zpool", bufs=2))
    small = ctx.enter_context(tc.tile_pool(name="small", bufs=2))
    ps_s = ctx.enter_context(tc.tile_pool(name="ps_s", bufs=1, space="PSUM"))
    ps_y = ctx.enter_context(tc.tile_pool(name="ps_y", bufs=2, space="PSUM"))
    ps_b = ctx.enter_context(tc.tile_pool(name="ps_b", bufs=2, space="PSUM"))

    # ---------------- resident loads ----------------
    inp_i = const.tile([128, NCH, D, BL], bf16)
    nc.sync.dma_start(out=inp_i[:], in_=inpi_d[:])
    wf = const.tile([128, NCH, D, JE], bf16)
    for ch in range(NCH):
        eng = nc.sync if ch % 2 == 0 else nc.scalar
        eng.dma_start(out=wf[:, ch], in_=wf_d[:, ch])
    inp_id = const.tile([128, NG, BL], bf16)
    nc.sync.dma_start(out=inp_id[:], in_=inpid_d[:])
    wm1a = const.tile([128, NG, 128], bf16)
    for h in range(4):
        nc.scalar.dma_start(out=wm1a[:, 32 * h:32 * (h + 1)],
                            in_=wm1a_d[:, 32 * h:32 * (h + 1)])
    wm1b = const.tile([32, NG, 128], bf16)
    nc.scalar.dma_start(out=wm1b[:], in_=wm1b_d[:])
    ones = const.tile([128, 2, 32], bf16)
    nc.scalar.dma_start(out=ones[:], in_=ones_d[:])
    epsb = const.tile([BL, 1], f32)
    nc.vector.memset(epsb[:], EPS)

    # persistent state
    blog = state.tile([128, NCH, J, BL], f32)   # routing logits, [i, (j,b)]
    cbuf = state.tile([128, NCH, J, BL], b16r)  # coupling coeffs c
    sbT2 = state.tile([BL, J, 32], f32)         # transposed s (+garbage cols)
    vT = state.tile([BL, J, 32], bf16)          # v in b-part (garbage cols=0)
    vblkA = state.tile([128, 8, BL], bf16)      # block-diag v, rows (j8,e)
    vblkB = state.tile([32, 2, BL], bf16)       # block-diag v, rows (j2,e)
    vout = state.tile([BL, J, E], f32)          # final v for output
    s2T = state.tile([BL, 5, 2, E], f32)
    nrmT = state.tile([BL, 5, 2], f32)
    sclT = state.tile([BL, 5, 2], f32)
    tmpT = state.tile([BL, 5, 2], f32)
    nc.vector.memset(vT[:], 0.0)
    nc.vector.memset(vblkA[:], 0.0)
    nc.vector.memset(vblkB[:], 0.0)

    def valid_view(tile_ap):
        """[BL, J, 32] -> strided [BL, 5, 2, 16] view of the valid e-cols.

        Valid cols of j=2q+jj sit at flat offset 64q + 48jj (steps 64/48/1),
        expressed as a step-3 slice over 16-wide chunks.
        """
        return tile_ap.rearrange("b j e -> b (j e)") \
            .rearrange("b (q c s) -> b q c s", q=5, c=4, s=16)[:, :, 0::3, :]

    def squash(iter0, final=False):
        """psA/psB diag -> (transpose) -> squash in b-part -> vblkA/vblkB.

        True s = 0.1*s_raw on iter0: n_true = 0.01*n_raw,
        v = squash_scale(n_true) * 0.1 * s_raw.
        """
        sAP = valid_view(sbT2[:])
        nc.scalar.square(s2T[:], sAP)
        nc.vector.tensor_reduce(nrmT[:], s2T[:], AX.X, ALU.add)
        k = 0.01 if iter0 else 1.0
        # tmpT = 1/(1 + k*n)
        nc.scalar.activation(tmpT[:], nrmT[:], AF.Copy, scale=k)
        nc.vector.tensor_scalar_add(tmpT[:], tmpT[:], 1.0)
        nc.vector.reciprocal(tmpT[:], tmpT[:])
        # sclT = 1/sqrt(k*n + eps)
        nc.scalar.activation(sclT[:], nrmT[:], AF.Sqrt, scale=k, bias=epsb[:])
        nc.vector.reciprocal(sclT[:], sclT[:])
        # sclT = k*n * tmpT * sclT * (0.1 iter0)
        nc.vector.tensor_mul(sclT[:], sclT[:], tmpT[:])
        kk = k * (0.1 if iter0 else 1.0)
        nc.scalar.activation(sclT[:], sclT[:], AF.Copy, scale=kk)
        nc.vector.tensor_mul(sclT[:], sclT[:], nrmT[:])
        scl_b = sclT[:].unsqueeze(3).broadcast_to([BL, 5, 2, 16])
        # vT = s * scale (broadcast over e), on the valid cols view only;
        # garbage cols of vT stay 0 so the back-transposes below write
        # exact block-diagonal vblk tiles.
        nc.vector.tensor_tensor(valid_view(vT[:]), sAP, scl_b, ALU.mult)
        if final:
            nc.vector.tensor_tensor(
                vout[:].rearrange("b (q c) e -> b q c e", q=5, c=2),
                sAP, scl_b, ALU.mult)
        # back-transposes: vT[:, j] is [32b x 32] with v_j at cols
        # 16*(j%2)+e, zeros elsewhere; its transpose is the [32, 32]
        # block-diagonal tile of vblk at rows 32*(j//2), col-block j.
        for j in range(8):
            q = j // 2
            nc.vector.transpose(
                vblkA[32 * q:32 * (q + 1), j], vT[:, j])
        nc.vector.transpose(vblkB[:, 0], vT[:, 8])
        nc.vector.transpose(vblkB[:, 1], vT[:, 9])

    def s_step(it):
        """cbuf (or uniform 0.1 if it==0) -> s matmuls -> psA/psB diag ->
        forward transposes into sbT2 (b-partition layout for squash)."""
        if it == 0:
            psA = ps_s.tile([128, 8, BL], f32, name="psA0", tag="psA")
            psB = ps_s.tile([32, 2, BL], f32, name="psB0", tag="psB")
            k = 0
            for ch in range(NCH):
                for d in range(D):
                    st, sp = (k == 0), (k == NCH * D - 1)
                    rhs = inp_i[:, ch, d]
                    nc.tensor.matmul(psA[:, 0], wf[:, ch, d, 0:128], rhs,
                                     start=st, stop=sp)
                    nc.tensor.matmul(psB[:, 0], wf[:, ch, d, 128:160], rhs,
                                     start=st, stop=sp)
                    k += 1
            for j in range(8):
                q = j // 2
                nc.vector.transpose(sbT2[:, j], psA[32 * q:32 * (q + 1), 0])
            nc.vector.transpose(sbT2[:, 8], psB[:, 0])
            nc.vector.transpose(sbT2[:, 9], psB[:, 0])
        else:
            psA = ps_s.tile([128, 8, BL], f32, name=f"psA{it}", tag="psA")
            psB = ps_s.tile([32, 2, BL], f32, name=f"psB{it}", tag="psB")
            k = 0
            for ch in range(NCH):
                X = xpool.tile([128, D, J, BL], bf16, name=f"X{it}_{ch}",
                               tag="X")
                cin = cbuf[:, ch].unsqueeze(1).broadcast_to([128, D, J, BL])
                iin = inp_i[:, ch].unsqueeze(2).broadcast_to([128, D, J, BL])
                nc.vector.tensor_tensor(X[:], cin, iin, ALU.mult)
                for d in range(D):
                    st, sp = (k == 0), (k == NCH * D - 1)
                    nc.tensor.matmul(
                        psA[:].rearrange("p j b -> p (j b)"),
                        wf[:, ch, d, 0:128],
                        X[:, d, 0:8].rearrange("p j b -> p (j b)"),
                        start=st, stop=sp)
                    nc.tensor.matmul(
                        psB[:].rearrange("p j b -> p (j b)"),
                        wf[:, ch, d, 128:160],
                        X[:, d, 8:10].rearrange("p j b -> p (j b)"),
                        start=st, stop=sp)
                    k += 1
            for j in range(8):
                q = j // 2
                nc.vector.transpose(sbT2[:, j],
                                    psA[32 * q:32 * (q + 1), j])
            nc.vector.transpose(sbT2[:, 8], psB[:, 0])
            nc.vector.transpose(sbT2[:, 9], psB[:, 1])

    def t_step(it):
        """vblk -> blog (it==0: overwrite; else accumulate) + softmax,
        pipelined per sup (sup == ch of blog/cbuf)."""
        for sup in range(NCH):
            bp = ps_b.tile([128, J, BL], f32, name=f"bp{it}_{sup}", tag="bp")
            for half in range(8 // CB):
                # 512 f32 per cc: pad to a full PSUM bank so no matmul
                # output crosses a bank boundary (writes past a bank edge
                # corrupt silently).
                yp = ps_y.tile([128, CB, 512], f32,
                               name=f"yp{it}_{sup}_{half}", tag="yp")
                for cc in range(CB):
                    g = sup * 8 + half * CB + cc
                    nc.tensor.matmul(
                        yp[:, cc, 0:256],
                        wm1a[:, g], vblkA[:].rearrange("p j b -> p (j b)"),
                        start=True, stop=True)
                    nc.tensor.matmul(
                        yp[:, cc, 256:320],
                        wm1b[:, g], vblkB[:].rearrange("p j b -> p (j b)"),
                        start=True, stop=True)
                ysb = ypool.tile([128, CB, J, BL], bf16,
                                 name=f"ysb{it}_{sup}_{half}", tag="ysb")
                nc.scalar.copy(
                    ysb[:].rearrange("p c j b -> p c (j b)"),
                    yp[:, :, 0:JB])
                Z = zpool.tile([128, CB, J, BL], bf16,
                               name=f"Z{it}_{sup}_{half}", tag="Z")
                g0 = sup * 8 + half * CB
                nc.vector.tensor_tensor(
                    Z[:], ysb[:],
                    inp_id[:, g0:g0 + CB].unsqueeze(2)
                    .broadcast_to([128, CB, J, BL]),
                    ALU.mult)
                for cc in range(CB):
                    gq = half * CB + cc
                    h, cg = gq % 2, gq // 2
                    # chunk gq's d-sum lands in bp rows [16gq,16gq+16);
                    # col-group-tiled [128,32] lhsT (constant per parity)
                    # keeps LDWEIGHTS at 32 cols and lets the 4 row-strip
                    # matmul streams overlap on different subarrays.
                    nc.tensor.matmul(
                        bp[32 * cg:32 * (cg + 1)]
                        .rearrange("p j b -> p (j b)"),
                        ones[:, h],
                        Z[:, cc].rearrange("p j b -> p (j b)"),
                        start=(h == 0), stop=(h == 1),
                        tile_position=(0, 32 * cg))
            if it == 0:
                nc.scalar.copy(blog[:, sup], bp[:])
            else:
                nc.vector.tensor_add(blog[:, sup], blog[:, sup], bp[:])
            # softmax over j for this sup (logits are small; no max-sub)
            nc.scalar.activation(cbuf[:, sup], blog[:, sup], AF.Exp)
            ssum = small.tile([128, BL], f32, name=f"ss{it}_{sup}",
                              tag="ssum")
            nc.vector.tensor_reduce(
                ssum[:], cbuf[:, sup].rearrange("p j b -> p b j"),
                AX.X, ALU.add)
            nc.vector.reciprocal(ssum[:], ssum[:])
            nc.vector.tensor_mul(
                cbuf[:, sup], cbuf[:, sup],
                ssum[:].unsqueeze(1).broadcast_to([128, J, BL]))

    # ---------------- the routing schedule ----------------
    dbg = _kernel_body.debug_tensors
    s_step(0)
    if "sbT2_0" in dbg:
        nc.sync.dma_start(out=dbg["sbT2_0"][:], in_=sbT2[:])
    squash(True)          # v0
    if "vblkA_0" in dbg:
        nc.sync.dma_start(out=dbg["vblkA_0"][:], in_=vblkA[:])
        nc.sync.dma_start(out=dbg["vblkB_0"][:], in_=vblkB[:])
    t_step(0)             # blog = t0, c1 = softmax(blog)
    if "blog_0" in dbg:
        nc.sync.dma_start(out=dbg["blog_0"][:], in_=blog[:])
        nc.sync.dma_start(out=dbg["cbuf_0"][:], in_=cbuf[:])
    s_step(1)
    if "sbT2_1" in dbg:
        nc.sync.dma_start(out=dbg["sbT2_1"][:], in_=sbT2[:])
    squash(False)         # v1
    t_step(1)             # blog += t1, c2 = softmax(blog)
    s_step(2)
    squash(False, final=True)  # v2 -> vout

    # ---------------- output ----------------
    nc.sync.dma_start(out=outv_d[:], in_=vout[:])
    nc.sync.dma_start(out=outc_d[:], in_=cbuf[:])


def kernel(inputs, W):
    global _PROGRAM
    from concourse.bass_utils import run_bass_kernel_spmd

    shared, per_core = _host_prep(np.asarray(inputs), np.asarray(W))
    if _PROGRAM is None:
        _PROGRAM = _build_program()
    in_maps = [{**shared, **pc} for pc in per_core]
    res = run_bass_kernel_spmd(_PROGRAM, in_maps, core_ids=list(range(NCORES)))
    outs = []
    for r in res.results:
        v = np.asarray(r["outv"], dtype=np.float32)        # [BL, J, E]
        c = np.asarray(r["outc"]).astype(np.float32)       # [128, NCH, J, BL]
        c = c.transpose(3, 2, 1, 0).reshape(BL, J, I)
        outs.append(np.concatenate([v, c], axis=-1))
    return np.concatenate(outs, axis=0).astype(np.float32)


if __name__ == "__main__":
    rng = np.random.default_rng(0)
    x = rng.standard_normal((B, I, D), dtype=np.float32)
    w = rng.standard_normal((J, I, E, D), dtype=np.float32)
    y = kernel(x, w)
    print(y.shape, y.dtype)


# revision 10
# speedup vs baseline: 15.3650x; 1.3166x over previous
"""CapsuleLayer dynamic-routing kernel for Trainium2 (8 NeuronCores), v2.

Problem: B=256, I=2048, D=8 input capsules -> J=10, E=16 output capsules,
3 routing iterations.  Output = concat([v2, c2], axis=-1) -> [B, J, E+I].

Sharding: pure data parallelism over batch (32 batches/core), W replicated.

v2 design (vs v1): bf16 matmul datapath, all weights SBUF-resident (both
layouts), ~24 large DMAs total, block-diagonal vblk for the t-step M1
(K=128 instead of 2560 K=16 matmuls), X=c*inputs on DVE, Y-copy on ACT,
chunk-pipelined t->softmax->X->s schedule.

Per-core steps (u_hat never materialized):
  s-step:  s[b,j,e] = sum_{i,d} X[b,j,i,d] W[j,i,e,d],  X = c (.) inputs
           diag trick: psA[(j8,e),(j8,b)] += wf[i,(j,e)]^T X[i,(j,b)]
           per (ch,d), PSUM-accumulated; psB for j=8,9.  it0: X==inputs
           (c uniform, 0.1 folded into squash), no diag needed: N=32.
  t-step:  M1: Y[(i,d)chunk,(j,b)] = wm1chunk[(j,e),(i,d)]^T vblk[(j,e),(j,b)]
           (vblk block-diagonal, built directly by the squash transposes)
           Z = Y (.) inputs (ACT copy PSUM->SBUF bf16, DVE multiply)
           M2: blog[i,(j,b)] += ones_blkdiag^T Z  (sums d)
  softmax over j without max-subtraction (logits are O(few)), per-sup
  pipelined right after its blog rows are produced.

Layouts (i = ch*128 + p; k = i*8 + d, g = k/128, q = k%128):
  wf    [128,16,8,160] bf16  wf[p,ch,d,16j+e] = W[j, 128ch+p, e, d]
  wm1a  [128,128,128]  bf16  wm1a[16j+e,g,c]  = W[j, (128g+c)/8, e, (128g+c)%8], j<8
  wm1b  [32,128,128]   bf16  same, j in {8,9}, row 16(j-8)+e
  ones  [128,8,128]    bf16  ones[q,gq,m] = (m == 16*gq + q//8)
  inp_i [128,16,8,32]  bf16  inp_i[p,ch,d,b] = x[b0+b, 128ch+p, d]
  inp_id[128,128,32]   bf16  inp_id[q,g,b]   = x[b0+b, 16g+q//8, q%8]
"""

import numpy as np

B, I, D, J, E = 256, 2048, 8, 10, 16
NCORES = 8
BL = B // NCORES          # 32 batches per core
NCH = I // 128            # 16 i-chunks of 128
NG = (I * D) // 128       # 128 (i,d)-groups of 128
JE = J * E                # 160
JB = J * BL               # 320
OUTW = E + I              # 2064
EPS = 1e-7
CB = 2                    # (i,d)-chunks per Y/Z batch in the t-step

_PROGRAM = None
_BUILD_DEBUG = False


def _host_prep(inputs, W):
    """Build all DRAM-side arrays. Returns (shared dict, per-core list)."""
    import concourse.mybir as mybir
    bf16 = mybir.dt.np(mybir.dt.float16)
    W = np.ascontiguousarray(W, dtype=np.float32)
    inputs = np.ascontiguousarray(inputs, dtype=np.float32)

    # wf[p, ch, d, 16j+e] = W[j, ch*128+p, e, d]
    wf = W.transpose(1, 3, 0, 2).reshape(NCH, 128, D, JE)
    wf = np.ascontiguousarray(wf.transpose(1, 0, 2, 3)).astype(bf16)

    # wm1[16j+e, (i,d)] = W[j, i, e, d], split j<8 / j>=8, grouped by 128
    wm1 = W.transpose(0, 2, 1, 3).reshape(J, E, I * D)
    wm1a = np.ascontiguousarray(
        wm1[0:8].reshape(128, NG, 128)).astype(bf16)
    wm1b = np.ascontiguousarray(
        wm1[8:10].reshape(32, NG, 128)).astype(bf16)

    # ones32[q, h, m] = 1 iff m == 16*h + q//8  (h = chunk parity)
    ones = np.zeros((128, 2, 32), dtype=np.float32)
    q = np.arange(128)
    for h in range(2):
        ones[q, h, 16 * h + q // 8] = 1.0
    ones = ones.astype(bf16)

    shared = {"wf": wf, "wm1a": wm1a, "wm1b": wm1b, "ones": ones}

    per_core = []
    for m in range(NCORES):
        sl = inputs[m * BL:(m + 1) * BL]                  # [32, 2048, 8]
        # inp_i[p, ch, d, b] = sl[b, ch*128+p, d]
        inp_i = np.ascontiguousarray(
            sl.reshape(BL, NCH, 128, D).transpose(2, 1, 3, 0)).astype(bf16)
        # inp_id[q, g, b] = sl[b, g*16 + q//8, q%8]
        inp_id = np.ascontiguousarray(
            sl.reshape(BL, NG, 16, 8).transpose(2, 3, 1, 0)
            .reshape(128, NG, BL)).astype(bf16)
        per_core.append({"inp_i": inp_i, "inp_id": inp_id})
    return shared, per_core


def _build_program():
    from contextlib import ExitStack
    import concourse.mybir as mybir
    from concourse import bacc
    from concourse.tile import TileContext

    f32 = mybir.dt.float32
    bf16 = mybir.dt.float16
    nc = bacc.Bacc()

    wf_d = nc.dram_tensor("wf", [128, NCH, D, JE], bf16, kind="ExternalInput")
    wm1a_d = nc.dram_tensor("wm1a", [128, NG, 128], bf16, kind="ExternalInput")
    wm1b_d = nc.dram_tensor("wm1b", [32, NG, 128], bf16, kind="ExternalInput")
    ones_d = nc.dram_tensor("ones", [128, 2, 32], bf16, kind="ExternalInput")
    inpi_d = nc.dram_tensor("inp_i", [128, NCH, D, BL], bf16,
                            kind="ExternalInput")
    inpid_d = nc.dram_tensor("inp_id", [128, NG, BL], bf16,
                             kind="ExternalInput")
    outv_d = nc.dram_tensor("outv", [BL, J, E], f32, kind="ExternalOutput")
    outc_d = nc.dram_tensor("outc", [128, NCH, J, BL], mybir.dt.bfloat16,
                            kind="ExternalOutput")

    _kernel_body.debug_tensors = {}
    if _BUILD_DEBUG:
        _kernel_body.debug_tensors = {
            "sbT2_0": nc.dram_tensor("dbg_sbT2_0", [BL, J, 32], f32,
                                     kind="ExternalOutput"),
            "vblkA_0": nc.dram_tensor("dbg_vblkA_0", [128, 8, BL], bf16,
                                      kind="ExternalOutput"),
            "vblkB_0": nc.dram_tensor("dbg_vblkB_0", [32, 2, BL], bf16,
                                      kind="ExternalOutput"),
            "blog_0": nc.dram_tensor("dbg_blog_0", [128, NCH, J, BL], f32,
                                     kind="ExternalOutput"),
            "cbuf_0": nc.dram_tensor("dbg_cbuf_0", [128, NCH, J, BL], mybir.dt.bfloat16,
                                     kind="ExternalOutput"),
            "sbT2_1": nc.dram_tensor("dbg_sbT2_1", [BL, J, 32], f32,
                                     kind="ExternalOutput"),
        }

    with ExitStack() as ctx:
        tc = ctx.enter_context(TileContext(nc))
        _kernel_body(ctx, tc, wf_d, wm1a_d, wm1b_d, ones_d, inpi_d, inpid_d,
                     outv_d, outc_d)
    nc.compile()
    return nc


def _kernel_body(ctx, tc, wf_d, wm1a_d, wm1b_d, ones_d, inpi_d, inpid_d,
                 outv_d, outc_d):
    import concourse.mybir as mybir

    f32 = mybir.dt.float32
    bf16 = mybir.dt.float16
    b16r = mybir.dt.bfloat16
    nc = tc.nc
    AF = mybir.ActivationFunctionType
    ALU = mybir.AluOpType
    AX = mybir.AxisListType

    # ---------------- pools ----------------
    const = ctx.enter_context(tc.tile_pool(name="const", bufs=1))
    state = ctx.enter_context(tc.tile_pool(name="state", bufs=1))
    xpool = ctx.enter_context(tc.tile_pool(name="xpool", bufs=2))
    ypool = ctx.enter_context(tc.tile_pool(name="ypool", bufs=2))
    zpool = ctx.enter_context(tc.tile_pool(name="zpool", bufs=2))
    small = ctx.enter_context(tc.tile_pool(name="small", bufs=2))
    ps_s = ctx.enter_context(tc.tile_pool(name="ps_s", bufs=1, space="PSUM"))
    ps_y = ctx.enter_context(tc.tile_pool(name="ps_y", bufs=2, space="PSUM"))
    ps_b = ctx.enter_context(tc.tile_pool(name="ps_b", bufs=2, space="PSUM"))

    # ---------------- resident loads ----------------
    inp_i = const.tile([128, NCH, D, BL], bf16)
    nc.sync.dma_start(out=inp_i[:], in_=inpi_d[:])
    wf = const.tile([128, NCH, D, JE], bf16)
    for ch in range(NCH):
        eng = nc.sync if ch % 2 == 0 else nc.scalar
        eng.dma_start(out=wf[:, ch], in_=wf_d[:, ch])
    inp_id = const.tile([128, NG, BL], bf16)
    nc.sync.dma_start(out=inp_id[:], in_=inpid_d[:])
    wm1a = const.tile([128, NG, 128], bf16)
    for h in range(4):
        nc.scalar.dma_start(out=wm1a[:, 32 * h:32 * (h + 1)],
                            in_=wm1a_d[:, 32 * h:32 * (h + 1)])
    wm1b = const.tile([32, NG, 128], bf16)
    nc.scalar.dma_start(out=wm1b[:], in_=wm1b_d[:])
    ones = const.tile([128, 2, 32], bf16)
    nc.scalar.dma_start(out=ones[:], in_=ones_d[:])
    epsb = const.tile([BL, 1], f32)
    nc.vector.memset(epsb[:], EPS)

    # persistent state
    blog = state.tile([128, NCH, J, BL], f32)   # routing logits, [i, (j,b)]
    cbuf = state.tile([128, NCH, J, BL], b16r)  # coupling coeffs c
    sbT2 = state.tile([BL, J, 32], f32)         # transposed s (+garbage cols)
    vT = state.tile([BL, J, 32], bf16)          # v in b-part (garbage cols=0)
    vblkA = state.tile([128, 8, BL], bf16)      # block-diag v, rows (j8,e)
    vblkB = state.tile([32, 2, BL], bf16)       # block-diag v, rows (j2,e)
    vout = state.tile([BL, J, E], f32)          # final v for output
    s2T = state.tile([BL, 5, 2, E], f32)
    nrmT = state.tile([BL, 5, 2], f32)
    sclT = state.tile([BL, 5, 2], f32)
    tmpT = state.tile([BL, 5, 2], f32)
    nc.vector.memset(vT[:], 0.0)
    nc.vector.memset(vblkA[:], 0.0)
    nc.vector.memset(vblkB[:], 0.0)

    def valid_view(tile_ap):
        """[BL, J, 32] -> strided [BL, 5, 2, 16] view of the valid e-cols.

        Valid cols of j=2q+jj sit at flat offset 64q + 48jj (steps 64/48/1),
        expressed as a step-3 slice over 16-wide chunks.
        """
        return tile_ap.rearrange("b j e -> b (j e)") \
            .rearrange("b (q c s) -> b q c s", q=5, c=4, s=16)[:, :, 0::3, :]

    def squash(iter0, final=False):
        """psA/psB diag -> (transpose) -> squash in b-part -> vblkA/vblkB.

        True s = 0.1*s_raw on iter0: n_true = 0.01*n_raw,
        v = squash_scale(n_true) * 0.1 * s_raw.
        """
        sAP = valid_view(sbT2[:])
        nc.scalar.square(s2T[:], sAP)
        nc.vector.tensor_reduce(nrmT[:], s2T[:], AX.X, ALU.add)
        k = 0.01 if iter0 else 1.0
        # tmpT = 1/(1 + k*n)
        nc.scalar.activation(tmpT[:], nrmT[:], AF.Copy, scale=k)
        nc.vector.tensor_scalar_add(tmpT[:], tmpT[:], 1.0)
        nc.vector.reciprocal(tmpT[:], tmpT[:])
        # sclT = 1/sqrt(k*n + eps)
        nc.scalar.activation(sclT[:], nrmT[:], AF.Sqrt, scale=k, bias=epsb[:])
        nc.vector.reciprocal(sclT[:], sclT[:])
        # sclT = k*n * tmpT * sclT * (0.1 iter0)
        nc.vector.tensor_mul(sclT[:], sclT[:], tmpT[:])
        kk = k * (0.1 if iter0 else 1.0)
        nc.scalar.activation(sclT[:], sclT[:], AF.Copy, scale=kk)
        nc.vector.tensor_mul(sclT[:], sclT[:], nrmT[:])
        scl_b = sclT[:].unsqueeze(3).broadcast_to([BL, 5, 2, 16])
        # vT = s * scale (broadcast over e), on the valid cols view only;
        # garbage cols of vT stay 0 so the back-transposes below write
        # exact block-diagonal vblk tiles.
        nc.vector.tensor_tensor(valid_view(vT[:]), sAP, scl_b, ALU.mult)
        if final:
            nc.vector.tensor_tensor(
                vout[:].rearrange("b (q c) e -> b q c e", q=5, c=2),
                sAP, scl_b, ALU.mult)
        # back-transposes: vT[:, j] is [32b x 32] with v_j at cols
        # 16*(j%2)+e, zeros elsewhere; its transpose is the [32, 32]
        # block-diagonal tile of vblk at rows 32*(j//2), col-block j.
        for j in range(8):
            q = j // 2
            nc.vector.transpose(
                vblkA[32 * q:32 * (q + 1), j], vT[:, j])
        nc.vector.transpose(vblkB[:, 0], vT[:, 8])
        nc.vector.transpose(vblkB[:, 1], vT[:, 9])

    def s_step(it):
        """cbuf (or uniform 0.1 if it==0) -> s matmuls -> psA/psB diag ->
        forward transposes into sbT2 (b-partition layout for squash)."""
        if it == 0:
            psA = ps_s.tile([128, 2, BL], f32, name="psA0", tag="psA")
            psB = ps_s.tile([32, 2, BL], f32, name="psB0", tag="psB")
            k = 0
            for ch in range(NCH):
                for d in range(D):
                    st, sp = (k == 0), (k == NCH * D - 1)
                    rhs = inp_i[:, ch, d]
                    nc.tensor.matmul(psA[:, 0], wf[:, ch, d, 0:128], rhs,
                                     start=st, stop=sp)
                    nc.tensor.matmul(psB[:, 0], wf[:, ch, d, 128:160], rhs,
                                     start=st, stop=sp)
                    k += 1
            for j in range(8):
                q = j // 2
                nc.vector.transpose(sbT2[:, j], psA[32 * q:32 * (q + 1), 0])
            nc.vector.transpose(sbT2[:, 8], psB[:, 0])
            nc.vector.transpose(sbT2[:, 9], psB[:, 0])
        else:
            psA = ps_s.tile([128, 2, BL], f32, name=f"psA{it}", tag="psA")
            psB = ps_s.tile([32, 2, BL], f32, name=f"psB{it}", tag="psB")
            k = 0
            for ch in range(NCH):
                X = xpool.tile([128, D, J, BL], bf16, name=f"X{it}_{ch}",
                               tag="X")
                cin = cbuf[:, ch].unsqueeze(1).broadcast_to([128, D, J, BL])
                iin = inp_i[:, ch].unsqueeze(2).broadcast_to([128, D, J, BL])
                nc.vector.tensor_tensor(X[:], cin, iin, ALU.mult)
                for d in range(D):
                    st, sp = (k == 0), (k == NCH * D - 1)
                    # col-group tiling: pair q's [32,32] weight strip and
                    # its own 64 moving cols; the 4 strips stream
                    # concurrently on different subarrays.
                    for q in range(4):
                        nc.tensor.matmul(
                            psA[32 * q:32 * (q + 1)]
                            .rearrange("p j b -> p (j b)"),
                            wf[:, ch, d, 32 * q:32 * (q + 1)],
                            X[:, d, 2 * q:2 * q + 2]
                            .rearrange("p j b -> p (j b)"),
                            start=st, stop=sp,
                            tile_position=(0, 32 * q))
                    nc.tensor.matmul(
                        psB[:].rearrange("p j b -> p (j b)"),
                        wf[:, ch, d, 128:160],
                        X[:, d, 8:10].rearrange("p j b -> p (j b)"),
                        start=st, stop=sp)
                    k += 1
            for j in range(8):
                q = j // 2
                nc.vector.transpose(sbT2[:, j],
                                    psA[32 * q:32 * (q + 1), j % 2])
            nc.vector.transpose(sbT2[:, 8], psB[:, 0])
            nc.vector.transpose(sbT2[:, 9], psB[:, 1])

    def t_step(it):
        """vblk -> blog (it==0: overwrite; else accumulate) + softmax,
        pipelined per sup (sup == ch of blog/cbuf)."""
        for sup in range(NCH):
            bp = ps_b.tile([128, J, BL], f32, name=f"bp{it}_{sup}", tag="bp")
            for half in range(8 // CB):
                # 512 f32 per cc: pad to a full PSUM bank so no matmul
                # output crosses a bank boundary (writes past a bank edge
                # corrupt silently).
                yp = ps_y.tile([128, CB, 512], f32,
                               name=f"yp{it}_{sup}_{half}", tag="yp")
                for cc in range(CB):
                    g = sup * 8 + half * CB + cc
                    nc.tensor.matmul(
                        yp[:, cc, 0:256],
                        wm1a[:, g], vblkA[:].rearrange("p j b -> p (j b)"),
                        start=True, stop=True)
                    nc.tensor.matmul(
                        yp[:, cc, 256:320],
                        wm1b[:, g], vblkB[:].rearrange("p j b -> p (j b)"),
                        start=True, stop=True)
                ysb = ypool.tile([128, CB, J, BL], bf16,
                                 name=f"ysb{it}_{sup}_{half}", tag="ysb")
                nc.scalar.copy(
                    ysb[:].rearrange("p c j b -> p c (j b)"),
                    yp[:, :, 0:JB])
                Z = zpool.tile([128, CB, J, BL], bf16,
                               name=f"Z{it}_{sup}_{half}", tag="Z")
                g0 = sup * 8 + half * CB
                nc.vector.tensor_tensor(
                    Z[:], ysb[:],
                    inp_id[:, g0:g0 + CB].unsqueeze(2)
                    .broadcast_to([128, CB, J, BL]),
                    ALU.mult)
                for cc in range(CB):
                    gq = half * CB + cc
                    h, cg = gq % 2, gq // 2
                    # chunk gq's d-sum lands in bp rows [16gq,16gq+16);
                    # col-group-tiled [128,32] lhsT (constant per parity)
                    # keeps LDWEIGHTS at 32 cols and lets the 4 row-strip
                    # matmul streams overlap on different subarrays.
                    nc.tensor.matmul(
                        bp[32 * cg:32 * (cg + 1)]
                        .rearrange("p j b -> p (j b)"),
                        ones[:, h],
                        Z[:, cc].rearrange("p j b -> p (j b)"),
                        start=(h == 0), stop=(h == 1),
                        tile_position=(0, 32 * cg))
            if it == 0:
                nc.scalar.copy(blog[:, sup], bp[:])
            else:
                nc.vector.tensor_add(blog[:, sup], blog[:, sup], bp[:])
            # softmax over j for this sup (logits are small; no max-sub)
            nc.scalar.activation(cbuf[:, sup], blog[:, sup], AF.Exp)
            ssum = small.tile([128, BL], f32, name=f"ss{it}_{sup}",
                              tag="ssum")
            nc.vector.tensor_reduce(
                ssum[:], cbuf[:, sup].rearrange("p j b -> p b j"),
                AX.X, ALU.add)
            nc.vector.reciprocal(ssum[:], ssum[:])
            nc.vector.tensor_mul(
                cbuf[:, sup], cbuf[:, sup],
                ssum[:].unsqueeze(1).broadcast_to([128, J, BL]))

    # ---------------- the routing schedule ----------------
    dbg = _kernel_body.debug_tensors
    s_step(0)
    if "sbT2_0" in dbg:
        nc.sync.dma_start(out=dbg["sbT2_0"][:], in_=sbT2[:])
    squash(True)          # v0
    if "vblkA_0" in dbg:
        nc.sync.dma_start(out=dbg["vblkA_0"][:], in_=vblkA[:])
        nc.sync.dma_start(out=dbg["vblkB_0"][:], in_=vblkB[:])
    t_step(0)             # blog = t0, c1 = softmax(blog)
    if "blog_0" in dbg:
        nc.sync.dma_start(out=dbg["blog_0"][:], in_=blog[:])
        nc.sync.dma_start(out=dbg["cbuf_0"][:], in_=cbuf[:])
    s_step(1)
    if "sbT2_1" in dbg:
        nc.sync.dma_start(out=dbg["sbT2_1"][:], in_=sbT2[:])
    squash(False)         # v1
    t_step(1)             # blog += t1, c2 = softmax(blog)
    s_step(2)
    squash(False, final=True)  # v2 -> vout

    # ---------------- output ----------------
    nc.sync.dma_start(out=outv_d[:], in_=vout[:])
    nc.sync.dma_start(out=outc_d[:], in_=cbuf[:])


def kernel(inputs, W):
    global _PROGRAM
    from concourse.bass_utils import run_bass_kernel_spmd

    shared, per_core = _host_prep(np.asarray(inputs), np.asarray(W))
    if _PROGRAM is None:
        _PROGRAM = _build_program()
    in_maps = [{**shared, **pc} for pc in per_core]
    res = run_bass_kernel_spmd(_PROGRAM, in_maps, core_ids=list(range(NCORES)))
    outs = []
    for r in res.results:
        v = np.asarray(r["outv"], dtype=np.float32)        # [BL, J, E]
        c = np.asarray(r["outc"]).astype(np.float32)       # [128, NCH, J, BL]
        c = c.transpose(3, 2, 1, 0).reshape(BL, J, I)
        outs.append(np.concatenate([v, c], axis=-1))
    return np.concatenate(outs, axis=0).astype(np.float32)


if __name__ == "__main__":
    rng = np.random.default_rng(0)
    x = rng.standard_normal((B, I, D), dtype=np.float32)
    w = rng.standard_normal((J, I, E, D), dtype=np.float32)
    y = kernel(x, w)
    print(y.shape, y.dtype)
